# revision 1
# baseline (speedup 1.0000x reference)
"""Trainium2 Bass kernel for nn_Choquet_integral (N_IN=12, N_OUT=16, M=16384).

Math (per input row x[0:12], fuzzy-measure table FM[4095, 16]):
    reference: sort x descending -> s, diffs_j = s_j - s_{j+1} (s_12 = 0),
    cumulative-set index c_j = sum_{t<=j} 2^{sortInd_t} - 1,
    out = sum_j diffs_j * FM[c_j].

Sort-free, scatter-free reformulation (Abel summation):
    maskA_i = sum_j 2^j * [pos_j <= pos_i]   (12-bit mask of elements ranked
                                              at-or-above element i, incl. i)
    maskB_i = maskA_i - 2^i
    out     = sum_i x_i * (T[maskA_i] - T[maskB_i])
where T[v] = FM[v-1] (v >= 1), T[0] = 0  -> T is a [4096, 16] table.

[pos_j <= pos_i] = [x_j > x_i] or (x_j == x_i and j <= i) (stable argsort
tie-break).  Computed as ONE contiguous is_ge per j against a host-prepared
comparand xz: x_j exact where i >= j, nextafter(x_j, -inf) where i < j
(a 1-ulp nudge turns >= into > exactly; it can only matter on exact ties).
maskA accumulates via fused (Q_j * 2^j) + acc scalar_tensor_tensor ops.

Table lookups: gpsimd ap_gather (SBUF-local, SIMD over the 16 partitions of
each Q7 core).  The table is stored transposed+replicated: partition 16c+o
holds T[:, o], so core c's gather of item t delivers all 16 output columns
of T[idx[t]] across its 16 partitions in one indexed column read.  This
forces the row->partition map m = c*(M/8) + g*16 + q <-> partition p=16c+q
(host-side row permutation, free); weights x_i arrive pre-broadcast (xr).

Item order t = (i*16+g)*16 + q (i outermost) so the i-reduction tree and
the gather-half splits are all contiguous slices.

EVERY compute op uses fully contiguous APs: this backend charges ~3.5us per
strided-AP segment and ~32ns/element on DVE, so segmented APs and scans are
ruinous (measured, not modeled).
"""

import numpy as np

import concourse.bacc as bacc
import concourse.bass as bass
import concourse.mybir as mybir
from concourse import library_config
from concourse.bass_utils import run_bass_kernel_spmd
from concourse.tile import TileContext

N_IN = 12
N_OUT = 16
N_VARS = 2**N_IN - 2  # 4094
M_FULL = 16384
N_CORES = 8
M_CORE = M_FULL // N_CORES  # 2048
NE = 2**N_IN  # 4096 table entries
F32 = mybir.dt.float32
I16 = mybir.dt.int16


# ---------------------------------------------------------------------------
# Host-side FM lattice build (exact numpy port of the reference _build_fm).
# ---------------------------------------------------------------------------
def _lattice_levels(n_in):
    levels = []
    for k in range(2, n_in):
        nodes = [s for s in range(1, 2**n_in - 1) if bin(s).count("1") == k]
        children = [
            [(s - (1 << b)) - 1 for b in range(n_in) if (s >> b) & 1] for s in nodes
        ]
        levels.append((np.array(nodes) - 1, np.array(children)))
    return levels


_LEVELS = _lattice_levels(N_IN)
_SINGLETONS = np.array([2**i - 1 for i in range(N_IN)])


def _build_table(fm_vars: np.ndarray) -> np.ndarray:
    """T_ext [4096, 16]: T_ext[0] = 0, T_ext[v] = FM[v-1]."""
    av = np.abs(fm_vars.astype(np.float32))
    FM = np.zeros((N_VARS, N_OUT), np.float32)
    FM[_SINGLETONS] = av[_SINGLETONS]
    for nodes_idx, children_idx in _LEVELS:
        mx = FM[children_idx].max(axis=1)
        FM[nodes_idx] = mx + av[nodes_idx]
    FM = np.concatenate([FM, np.ones((1, N_OUT), np.float32)], axis=0)
    FM = np.minimum(FM, np.float32(1.0))
    return np.concatenate([np.zeros((1, N_OUT), np.float32), FM], axis=0)


def build_bass(m_core: int = M_CORE, repeat: int = 1) -> bass.Bass:
    assert m_core % 128 == 0
    G = m_core // 128  # row-groups per partition
    n = N_IN  # 12
    nc_ = G * 16  # item columns per partition (= G*16 = rows_per_core)
    ncol = n * G  # idx columns (i-major: col = i*G... see below)
    ni = nc_ * n  # gathered items per Q7 core
    nh = ni // 2
    nc = bacc.Bacc()

    # col = i*G + g  (i-major);  item t = col*16 + q.
    # xz: [12 j-blocks of ncol] comparands; xe: x at (i,g); pw: 2^i at (i,g);
    # xr: per-item weights x_i at t-order, replicated across each core's
    # 16 partitions.
    t_d = nc.declare_dram_parameter("t", [128, NE], F32, isOutput=False)
    xz_d = nc.declare_dram_parameter("xz", [128, n * ncol], F32, isOutput=False)
    xe_d = nc.declare_dram_parameter("xe", [128, 2 * ncol], F32, isOutput=False)
    xr_d = nc.declare_dram_parameter("xr", [128, ni], F32, isOutput=False)
    y_d = nc.declare_dram_parameter("y", [128, nc_], F32, isOutput=True)

    add = mybir.AluOpType.add
    mult = mybir.AluOpType.mult

    with TileContext(nc) as tc:
        with tc.tile_pool(name="sbuf", bufs=1) as pool:
            t_sb = pool.tile([128, NE], F32)
            xz_sb = pool.tile([128, n * ncol], F32)
            xe_sb = pool.tile([128, 2 * ncol], F32)  # [xe | pw]
            xr_sb = pool.tile([128, ni], F32)
            q_sb = pool.tile([128, ncol], F32)
            acc_sb = pool.tile([128, ncol], F32)
            ia_sb = pool.tile([128, ncol], I16)
            ib_sb = pool.tile([128, ncol], I16)
            ga_sb = pool.tile([128, ni], F32)
            gb_sb = pool.tile([128, ni], F32)
            v_sb = pool.tile([128, ni], F32)
            t1_sb = pool.tile([128, ni // 2], F32)
            t2_sb = pool.tile([128, 2 * nc_], F32)
            o_sb = pool.tile([128, nc_], F32)

            nc.gpsimd.load_library(library_config.ap_gather)

            for _rep in range(repeat):
                # Loads: xz/xe on SP ring; t/xr on ACT ring.
                nc.sync.dma_start(out=xz_sb[:, :], in_=xz_d[:, :])
                nc.sync.dma_start(out=xe_sb[:, :], in_=xe_d[:, :])
                nc.scalar.dma_start(out=t_sb[:, :], in_=t_d[:, :])
                nc.scalar.dma_start(out=xr_sb[:, :], in_=xr_d[:, :])

                xe = xe_sb[:, 0:ncol]
                pw = xe_sb[:, ncol : 2 * ncol]

                # maskA accumulation: acc = sum_j 2^j * [x_j "(>=|>)" x_i]
                for j in range(n):
                    xzj = xz_sb[:, j * ncol : (j + 1) * ncol]
                    nc.vector.tensor_tensor(
                        out=q_sb[:, :], in0=xzj, in1=xe,
                        op=mybir.AluOpType.is_ge,
                    )
                    if j == 0:
                        nc.vector.tensor_copy(out=acc_sb[:, :], in_=q_sb[:, :])
                    else:
                        nc.vector.scalar_tensor_tensor(
                            out=acc_sb[:, :],
                            in0=q_sb[:, :],
                            scalar=float(1 << j),
                            in1=acc_sb[:, :],
                            op0=mult,
                            op1=add,
                        )

                # idxA = maskA (int16); idxB = maskA - 2^i (int16)
                nc.vector.tensor_copy(out=ia_sb[:, :], in_=acc_sb[:, :])
                nc.vector.scalar_tensor_tensor(
                    out=ib_sb[:, :],
                    in0=acc_sb[:, :],
                    scalar=1.0,
                    in1=pw,
                    op0=mult,
                    op1=mybir.AluOpType.subtract,
                )

                # Gathers (halves i<6 / i>=6, pipelined with the combine):
                def gather(out_tile, idx_tile, h):
                    nc.gpsimd.ap_gather(
                        out_ap=out_tile[:, h * nh : (h + 1) * nh],
                        in_ap=t_sb[:, :],
                        idxs_ap=idx_tile[:, h * ncol // 2 : (h + 1) * ncol // 2],
                        channels=128,
                        num_elems=NE,
                        d=1,
                        num_idxs=nh,
                    )

                gather(ga_sb, ia_sb, 0)
                gather(gb_sb, ib_sb, 0)
                gather(ga_sb, ia_sb, 1)
                gather(gb_sb, ib_sb, 1)

                # Per-half combine: V = (G_A - G_B) * xr, then i-tree.
                # t = i*(16*nc_/16...) : i-blocks of 16*G*16/..= nc_*16/16;
                # block size per i = nc_*16/16 = G*256/16... = nc_*16? No:
                # per i: 16*G columns * 16 q = 16*G*16 = nh/3... it's ni/12.
                bi = ni // n  # elements per i-block
                for h in range(2):
                    sl = slice(h * nh, (h + 1) * nh)
                    nc.vector.tensor_tensor(
                        out=v_sb[:, sl], in0=ga_sb[:, sl], in1=gb_sb[:, sl],
                        op=mybir.AluOpType.subtract,
                    )
                    nc.vector.tensor_tensor(
                        out=v_sb[:, sl], in0=v_sb[:, sl], in1=xr_sb[:, sl],
                        op=mult,
                    )
                    # 6 i-blocks -> 3 -> 1
                    b0 = h * nh
                    nc.vector.tensor_tensor(
                        out=t1_sb[:, h * nh // 2 : h * nh // 2 + 3 * bi],
                        in0=v_sb[:, b0 : b0 + 3 * bi],
                        in1=v_sb[:, b0 + 3 * bi : b0 + 6 * bi],
                        op=add,
                    )
                    tb = h * nh // 2
                    nc.vector.tensor_tensor(
                        out=t2_sb[:, h * nc_ : (h + 1) * nc_],
                        in0=t1_sb[:, tb : tb + bi],
                        in1=t1_sb[:, tb + bi : tb + 2 * bi],
                        op=add,
                    )
                    nc.vector.tensor_tensor(
                        out=t2_sb[:, h * nc_ : (h + 1) * nc_],
                        in0=t2_sb[:, h * nc_ : (h + 1) * nc_],
                        in1=t1_sb[:, tb + 2 * bi : tb + 3 * bi],
                        op=add,
                    )
                # final: o = half0 + half1
                nc.vector.tensor_tensor(
                    out=o_sb[:, :], in0=t2_sb[:, 0:nc_], in1=t2_sb[:, nc_:],
                    op=add,
                )

                # Store device-layout output [128, G*16]; host un-permutes.
                nc.sync.dma_start(out=y_d[:, :], in_=o_sb[:, :])

    nc.compile()
    return nc


_NC_CACHE: dict[tuple, bass.Bass] = {}


def _get_nc(m_core: int, repeat: int = 1) -> bass.Bass:
    key = (m_core, repeat)
    if key not in _NC_CACHE:
        _NC_CACHE[key] = build_bass(m_core, repeat)
    return _NC_CACHE[key]


def _prep_core_inputs(x_shard: np.ndarray, t_rep: np.ndarray) -> dict:
    """Host-side input prep.  Row m = c*(m_core//8) + g*16 + q lives on
    partition p = 16c+q, group g; item column col = i*G + g."""
    m_core = x_shard.shape[0]
    G = m_core // 128
    ncol = N_IN * G
    # x5[c, g, q, i]
    x5 = x_shard.reshape(8, G, 16, N_IN).astype(np.float32)
    # xe[p=16c+q, i*G+g] = x5[c, g, q, i]
    xe = x5.transpose(0, 2, 3, 1).reshape(8 * 16, ncol)  # [c,q | i,g]
    pw = np.broadcast_to(
        np.repeat((2.0 ** np.arange(N_IN)).astype(np.float32), G), (128, ncol)
    )
    xepw = np.concatenate([xe, pw], axis=1)
    # xz[p, j*ncol + i*G+g] = x_j (nudged down where i < j)
    xj = x5.transpose(0, 2, 3, 1)  # [c, q, j, g]
    xz = np.empty((8, 16, N_IN, N_IN, G), np.float32)  # [c, q, j, i, g]
    xz[:] = xj[:, :, :, None, :]
    dn = np.nextafter(xj, -np.inf)
    ii = np.arange(N_IN)
    lower = ii[None, :] > ii[:, None]  # [j, i]: i < j
    xz[:, :, lower] = np.broadcast_to(
        dn[:, :, :, None, :], xz.shape
    )[:, :, lower]
    xz = xz.reshape(128, N_IN * ncol)
    # xr[16c+o, (i*G+g)*16+q] = x5[c, g, q, i]  (replicated over o)
    xr = x5.transpose(0, 3, 1, 2).reshape(8, 1, -1)  # [c | i,g,q]
    xr = np.broadcast_to(xr, (8, 16, N_IN * G * 16)).reshape(128, -1)
    return {
        "t": t_rep,
        "xz": np.ascontiguousarray(xz),
        "xe": np.ascontiguousarray(xepw),
        "xr": np.ascontiguousarray(xr),
    }


def _post_core_output(y_dev: np.ndarray, m_core: int) -> np.ndarray:
    # y_dev [128, G*16]: [16c+o, g*16+q] -> y[c*(m_core//8)+g*16+q, o]
    G = m_core // 128
    y = y_dev.reshape(8, 16, G, 16)  # [c, o, g, q]
    y = y.transpose(0, 2, 3, 1)  # [c, g, q, o]
    return np.ascontiguousarray(y.reshape(m_core, 16))


def kernel(inputs: np.ndarray, fm_vars: np.ndarray, _repeat: int = 1) -> np.ndarray:
    inputs = np.ascontiguousarray(np.asarray(inputs, dtype=np.float32))
    fm_vars = np.asarray(fm_vars, dtype=np.float32)
    assert inputs.shape == (M_FULL, N_IN), inputs.shape
    table = _build_table(fm_vars)  # [4096, 16]
    t_rep = np.ascontiguousarray(np.tile(table.T, (8, 1)))  # [128, 4096]

    nc = _get_nc(M_CORE, _repeat)
    shards = inputs.reshape(N_CORES, M_CORE, N_IN)
    in_maps = [_prep_core_inputs(shards[c], t_rep) for c in range(N_CORES)]
    res = run_bass_kernel_spmd(nc, in_maps, list(range(N_CORES)))
    out = np.concatenate(
        [_post_core_output(r["y"], M_CORE) for r in res.results], axis=0
    )
    return out.astype(np.float32)

